# revision 12
# baseline (speedup 1.0000x reference)
"""Trainium2 Bass kernel for 3-layer GAT + inner-product decode + Student-t assignment.

Sharding (8 NeuronCores, SPMD, one program):
  - Nodes sharded row-wise: core c owns dst rows [c*1536, (c+1)*1536).
  - Per layer: each core computes H_aug = X @ [W | W@a_s | W@a_d] for its shard,
    AllGathers the augmented table (bf16) into every core's HBM, then
    dma_gathers per-edge rows (edges pre-sorted by dst on host, padded to a
    uniform per-dst-tile edge-tile count so all cores run the same program).
  - Edge softmax: u_e = exp(leaky_relu(as[src]+ad[dst])) batched; segment sums
    via one-hot B0 matrices (is_equal vs an iota row) feeding TensorE matmuls
    accumulating [dst, feat] in PSUM; softmax normalization is a per-partition
    reciprocal scale at eviction; ELU applied after a PE transpose so bias is
    per-partition.
  - Decode: sigmoid(z_blk @ z_full^T) tiled [128,512]; Student-t q via one
    augmented matmul (rows: z^2 | z | 1).
"""
import sys
sys.path.insert(0, '/opt/trn_rl_repo')
sys.path.insert(0, '/opt/trn_rl_repo/concourse')

import numpy as np

import concourse.bacc as bacc
import concourse.mybir as mybir
import concourse.tile as tile
from concourse.bass_utils import run_bass_kernel_spmd
from concourse.masks import make_identity

# ---- problem constants ----
N, E = 12288, 196608
IN_DIM, HID, HEADS, EMB, K = 128, 256, 4, 16, 2
NEG_SLOPE = 0.2
NC = 8
NS = N // NC            # 1536 nodes per shard
NT = NS // 128          # 12 dst tiles per shard
ROW = 1152              # bf16 elems per layer-1/2 table row (2304B, %256==0)
ZROW = 128              # layer-3 table row (bf16, 256B)
EPC = 512               # edges per dma_gather call

F32 = mybir.dt.float32
BF16 = mybir.dt.bfloat16
I16 = mybir.dt.int16


# ---------------- host-side preprocessing ----------------

def _prep_graph(edge_index):
    src = np.concatenate([edge_index[0], np.arange(N, dtype=np.int64)])
    dst = np.concatenate([edge_index[1], np.arange(N, dtype=np.int64)])
    order = np.argsort(dst, kind="stable")
    src, dst = src[order], dst[order]

    core = dst // NS
    tile_of = (dst % NS) // 128
    seg = np.zeros((NC, NT), np.int64)
    np.add.at(seg, (core, tile_of), 1)
    ET = int(np.ceil(seg.max() / 128))
    G = NT * ET * 128

    src_pad = np.zeros((NC, G), np.int64)
    dst_pad = np.zeros((NC, G), np.int64)
    dl_pad = np.full((NC, G), -1.0, np.float32)

    for c in range(NC):
        m = core == c
        s_c, d_c, t_c = src[m], dst[m], tile_of[m]
        for t in range(NT):
            mt = t_c == t
            cnt = int(mt.sum())
            base = t * ET * 128
            src_pad[c, base:base + cnt] = s_c[mt]
            dst_pad[c, base:base + cnt] = d_c[mt]
            dl_pad[c, base:base + cnt] = (d_c[mt] % NS) - t * 128
    return ET, G, src_pad, dst_pad, dl_pad


def _idx16(idx, G):
    out = np.zeros((16, G // 16), np.int16)
    j = np.arange(G)
    out[j % 16, j // 16] = idx
    return np.tile(out, (8, 1))


def _dlT(dl, G):
    out = np.zeros((128, G // 128), np.float32)
    j = np.arange(G)
    out[j % 128, j // 128] = dl
    return out


def _aug_w(W, a_s, a_d):
    Kdim, Fout = W.shape
    H = a_s.shape[0]
    F = Fout // H
    was = np.stack([W[:, h * F:(h + 1) * F] @ a_s[h] for h in range(H)], axis=1)
    wad = np.stack([W[:, h * F:(h + 1) * F] @ a_d[h] for h in range(H)], axis=1)
    return np.concatenate([W, was, wad], axis=1)


# ---------------- device program ----------------

def _build(nc, ET, G):
    NCALLS = G // EPC

    # ---- inputs ----
    xT = nc.dram_tensor("xT", [128, NS], BF16, kind="ExternalInput")
    W1a = nc.dram_tensor("W1a", [128, 1032], BF16, kind="ExternalInput")
    W2a = nc.dram_tensor("W2a", [128, 8, 1032], BF16, kind="ExternalInput")
    W3a = nc.dram_tensor("W3a", [128, 8, 18], BF16, kind="ExternalInput")
    b1c = nc.dram_tensor("b1c", [128, 8], F32, kind="ExternalInput")
    b2c = nc.dram_tensor("b2c", [128, 8], F32, kind="ExternalInput")
    cent = nc.dram_tensor("cent", [128, K], F32, kind="ExternalInput")
    srci = nc.dram_tensor("srci", [128, G // 16], I16, kind="ExternalInput")
    dsti = nc.dram_tensor("dsti", [128, G // 16], I16, kind="ExternalInput")
    dli = nc.dram_tensor("dli", [128, G // 128], F32, kind="ExternalInput")

    # ---- outputs ----
    z_out = nc.dram_tensor("z_out", [NS, EMB], F32, kind="ExternalOutput")
    adj_out = nc.dram_tensor("adj_out", [NS, N], F32, kind="ExternalOutput")
    q_out = nc.dram_tensor("q_out", [NS, K], F32, kind="ExternalOutput")

    # ---- internal DRAM ----
    hin = nc.dram_tensor("hin", [NS, ROW], BF16, kind="Internal")
    htab = nc.dram_tensor("htab", [N, ROW], BF16, kind="Internal", addr_space="Shared")
    zin = nc.dram_tensor("zin", [NS, ZROW], BF16, kind="Internal")
    ztab = nc.dram_tensor("ztab", [N, ZROW], BF16, kind="Internal", addr_space="Shared")
    ztin = nc.dram_tensor("ztin", [EMB, NS], F32, kind="Internal")
    zttab = nc.dram_tensor("zttab", [NC * EMB, NS], F32, kind="Internal", addr_space="Shared")
    xtd = [nc.dram_tensor(f"xtd{i}", [128, 8, NS], BF16, kind="Internal") for i in range(2)]

    with tile.TileContext(nc) as tc:
        with (
            tc.tile_pool(name="const", bufs=1) as cpool,
            tc.tile_pool(name="wpool", bufs=1) as wpool,
            tc.tile_pool(name="lhs", bufs=2) as lhs_pool,
            tc.tile_pool(name="hst", bufs=2) as hst_pool,
            tc.tile_pool(name="gath", bufs=6) as gpool,
            tc.tile_pool(name="dgath", bufs=6) as dpool,
            tc.tile_pool(name="alph", bufs=2) as apool,
            tc.tile_pool(name="b0p", bufs=6) as bpool,
            tc.tile_pool(name="ev", bufs=2) as epool,
            tc.tile_pool(name="sml", bufs=4) as spool,
            tc.tile_pool(name="psA", bufs=1, space="PSUM") as psA,
            tc.tile_pool(name="psS", bufs=1, space="PSUM") as psS,
            tc.tile_pool(name="psH", bufs=1, space="PSUM") as psH,
            tc.tile_pool(name="psT", bufs=1, space="PSUM") as psT,
        ):
            # constants
            iota_i = cpool.tile([128, 128], mybir.dt.int32)
            nc.gpsimd.iota(iota_i[:], pattern=[[1, 128]], base=0, channel_multiplier=0)
            iota_f = cpool.tile([128, 128], F32)
            nc.vector.tensor_copy(out=iota_f[:], in_=iota_i[:])
            ident = cpool.tile([128, 128], F32)
            make_identity(nc, ident[:])

            idx_s = cpool.tile([128, G // 16], I16)
            nc.sync.dma_start(out=idx_s[:], in_=srci[:])
            idx_d = cpool.tile([128, G // 16], I16)
            nc.sync.dma_start(out=idx_d[:], in_=dsti[:])
            dlt = cpool.tile([128, G // 128], F32)
            nc.sync.dma_start(out=dlt[:], in_=dli[:])

            w1 = wpool.tile([128, 1032], BF16)
            nc.sync.dma_start(out=w1[:], in_=W1a[:])
            w2 = wpool.tile([128, 8, 1032], BF16)
            nc.sync.dma_start(out=w2[:], in_=W2a[:])
            w3 = wpool.tile([128, 8, 18], BF16)
            nc.sync.dma_start(out=w3[:], in_=W3a[:])
            b1 = wpool.tile([128, 8], F32)
            nc.sync.dma_start(out=b1[:], in_=b1c[:])
            b2 = wpool.tile([128, 8], F32)
            nc.sync.dma_start(out=b2[:], in_=b2c[:])
            cent_s = wpool.tile([128, K], F32)
            nc.sync.dma_start(out=cent_s[:], in_=cent[:])

            x0 = wpool.tile([128, NS], BF16)
            nc.sync.dma_start(out=x0[:], in_=xT[:])

            zsb = wpool.tile([128, NT, EMB], F32)   # node-major z
            ztsb = wpool.tile([128, NS], F32)       # feature-major z (rows 0:16)

            # ================= three GAT layers =================
            for lyr in range(3):
                if lyr == 0:
                    kt, wsb, fout, row, tab, hin_l, H = 1, w1, 1024, ROW, htab, hin, HEADS
                elif lyr == 1:
                    kt, wsb, fout, row, tab, hin_l, H = 8, w2, 1024, ROW, htab, hin, HEADS
                else:
                    kt, wsb, fout, row, tab, hin_l, H = 8, w3, EMB, ZROW, ztab, zin, 1
                F = fout // H
                naux = fout + 2 * H
                asz = H

                # ---- phase A: H_aug = X @ W_aug ----
                for nt in range(NT):
                    if lyr == 0:
                        lhsts = [x0[:, nt * 128:(nt + 1) * 128]]
                    else:
                        lt = lhs_pool.tile([128, 8, 128], BF16, tag="lhs")
                        nc.sync.dma_start(out=lt[:], in_=xtd[(lyr + 1) % 2][:, :, nt * 128:(nt + 1) * 128])
                        lhsts = [lt[:, kk, :] for kk in range(8)]
                    ph = psH.tile([128, 1536], F32, tag="psH")
                    chunks = [(0, 512), (512, 512), (1024, naux - 1024)] if naux > 1024 else [(0, naux)]
                    for c0, cw in chunks:
                        for kk in range(kt):
                            nc.tensor.matmul(
                                out=ph[:, c0:c0 + cw],
                                lhsT=lhsts[kk],
                                rhs=(wsb[:, c0:c0 + cw] if kt == 1 else wsb[:, kk, c0:c0 + cw]),
                                start=(kk == 0), stop=(kk == kt - 1),
                            )
                    hs = hst_pool.tile([128, row], BF16, tag="hst")
                    nc.vector.tensor_copy(out=hs[:, 0:naux], in_=ph[:, 0:naux])
                    nc.sync.dma_start(
                        out=hin_l.rearrange("(t p) c -> p t c", p=128)[:, nt, :],
                        in_=hs[:],
                    )
                nc.gpsimd.collective_compute(
                    "AllGather", mybir.AluOpType.bypass,
                    replica_groups=[list(range(NC))],
                    ins=[hin_l[:]], outs=[tab[:]],
                )

                # ---- phase B: gather + alpha + segment aggregation ----
                u_all = apool.tile([128, NCALLS * 4 * asz], F32, tag="uall")
                u_bf = apool.tile([128, NCALLS * 4 * asz], BF16, tag="ubf")
                gts = {}
                for q in range(NCALLS):
                    gt = gpool.tile([128, EPC // 128, row], BF16, tag="gath")
                    gts[q] = gt
                    nc.gpsimd.dma_gather(
                        gt[:], tab[:],
                        idx_s[:, q * (EPC // 16):(q + 1) * (EPC // 16)],
                        EPC, EPC, row,
                    )
                    dgt = dpool.tile([128, EPC // 128, ZROW], BF16, tag="dgath")
                    if lyr < 2:
                        nc.gpsimd.dma_gather(
                            dgt[:], tab[:, fout:fout + ZROW],
                            idx_d[:, q * (EPC // 16):(q + 1) * (EPC // 16)],
                            EPC, EPC, ZROW, elem_step=row,
                        )
                        asv = gt[:, :, fout:fout + asz]
                        adv = dgt[:, :, asz:2 * asz]
                    else:
                        nc.gpsimd.dma_gather(
                            dgt[:], tab[:],
                            idx_d[:, q * (EPC // 16):(q + 1) * (EPC // 16)],
                            EPC, EPC, ZROW,
                        )
                        asv = gt[:, :, fout:fout + asz]
                        adv = dgt[:, :, fout + asz:fout + 2 * asz]
                    w = 4 * asz
                    ec = apool.tile([128, 4, asz], F32, tag="ecol")
                    nc.vector.tensor_tensor(out=ec[:], in0=asv, in1=adv,
                                            op=mybir.AluOpType.add)
                    e2 = apool.tile([128, 4, asz], F32, tag="e2")
                    nc.vector.tensor_scalar_mul(e2[:], ec[:], NEG_SLOPE)
                    nc.vector.tensor_tensor(out=e2[:], in0=ec[:], in1=e2[:],
                                            op=mybir.AluOpType.max)
                    nc.scalar.activation(
                        out=u_all[:, q * w:(q + 1) * w].rearrange("p (a b) -> p a b", b=asz),
                        in_=e2[:], func=mybir.ActivationFunctionType.Exp)
                    nc.vector.tensor_copy(out=u_bf[:, q * w:(q + 1) * w],
                                          in_=u_all[:, q * w:(q + 1) * w])

                for t in range(NT):
                    pa = psA.tile([128, max(fout, 32)], F32, tag="psA")
                    ps = psS.tile([128, 4], F32, tag="psS")
                    for j in range(ET):
                        k = t * ET + j
                        q, s = k // 4, k % 4
                        gt = gts[q]
                        b0 = bpool.tile([128, 128], BF16, tag="b0")
                        nc.vector.tensor_tensor(
                            out=b0[:], in0=dlt[:, k:k + 1].to_broadcast([128, 128]),
                            in1=iota_f[:], op=mybir.AluOpType.is_equal)
                        for h in range(H):
                            nc.vector.tensor_scalar_mul(
                                gt[:, s, h * F:(h + 1) * F],
                                gt[:, s, h * F:(h + 1) * F],
                                u_all[:, k * asz + h:k * asz + h + 1])
                        nc.tensor.matmul(
                            out=ps[:, 0:asz], lhsT=b0[:],
                            rhs=u_bf[:, k * asz:(k + 1) * asz],
                            start=(j == 0), stop=(j == ET - 1))
                        for c0 in range(0, fout, 512):
                            cw = min(512, fout - c0)
                            nc.tensor.matmul(
                                out=pa[:, c0:c0 + cw], lhsT=b0[:],
                                rhs=gt[:, s, c0:c0 + cw],
                                start=(j == 0), stop=(j == ET - 1))
                    rs = spool.tile([128, 4], F32, tag="rs")
                    nc.vector.reciprocal(out=rs[:, 0:asz], in_=ps[:, 0:asz])
                    an = epool.tile([128, max(fout, 32)], F32, tag="aggn")
                    for h in range(H):
                        nc.vector.tensor_scalar_mul(
                            an[:, h * F:(h + 1) * F], pa[:, h * F:(h + 1) * F],
                            rs[:, h:h + 1])
                    if lyr < 2:
                        bias = b1 if lyr == 0 else b2
                        xnew = xtd[lyr % 2]
                        xstage = epool.tile([128, 8, 128], BF16, tag="xstage")
                        for ch in range(8):
                            pt = psT.tile([128, 128], F32, tag="psT")
                            nc.tensor.transpose(
                                out=pt[:], in_=an[:, ch * 128:(ch + 1) * 128],
                                identity=ident[:])
                            # elu(v+b) = v+b if >0 else exp(v+b)-1
                            texp = spool.tile([128, 128], F32, tag="texp")
                            nc.scalar.activation(
                                out=texp[:], in_=pt[:],
                                func=mybir.ActivationFunctionType.Exp,
                                bias=bias[:, ch:ch + 1])
                            nc.vector.tensor_scalar_add(texp[:], texp[:], -1.0)
                            tlin = spool.tile([128, 128], F32, tag="tlin")
                            nc.vector.tensor_scalar(
                                out=tlin[:], in0=pt[:], scalar1=bias[:, ch:ch + 1],
                                scalar2=None, op0=mybir.AluOpType.add)
                            mask = spool.tile([128, 128], mybir.dt.uint8, tag="mask")
                            nc.vector.tensor_scalar(
                                out=mask[:], in0=tlin[:], scalar1=0.0,
                                scalar2=None, op0=mybir.AluOpType.is_gt)
                            nc.vector.select(
                                out=xstage[:, ch, :], mask=mask[:],
                                on_true=tlin[:], on_false=texp[:])
                        nc.sync.dma_start(
                            out=xnew[:, :, t * 128:(t + 1) * 128], in_=xstage[:])
                    else:
                        # z layer (b3 asserted zero host-side)
                        nc.vector.tensor_copy(out=zsb[:, t, :], in_=an[:, 0:EMB])
                        pt = psT.tile([128, 128], F32, tag="psT")
                        nc.tensor.transpose(
                            out=pt[0:EMB, :], in_=an[:, 0:EMB], identity=ident[:])
                        nc.vector.tensor_copy(
                            out=ztsb[0:EMB, t * 128:(t + 1) * 128], in_=pt[0:EMB, :])

            # ================= decode =================
            nc.sync.dma_start(out=z_out.rearrange("(t p) c -> p t c", p=128), in_=zsb[:])
            nc.sync.dma_start(out=ztin[:], in_=ztsb[0:EMB, :])
            nc.gpsimd.collective_compute(
                "AllGather", mybir.AluOpType.bypass,
                replica_groups=[list(range(NC))],
                ins=[ztin[:]], outs=[zttab[:]],
            )
            ztf = wpool.tile([EMB, N], F32)
            nc.sync.dma_start(
                out=ztf[:].rearrange("e (r n) -> e r n", r=NC),
                in_=zttab.rearrange("(r e) n -> e r n", e=EMB))

            # Student-t q: one matmul with lhsT rows [z^2 (0:16) | z (32:48) | 1 (64)]
            ztaug = wpool.tile([65, NS], F32)
            nc.vector.memset(ztaug[:], 0.0)
            nc.vector.tensor_tensor(out=ztaug[0:EMB, :], in0=ztsb[0:EMB, :],
                                    in1=ztsb[0:EMB, :], op=mybir.AluOpType.mult)
            nc.vector.tensor_copy(out=ztaug[32:32 + EMB, :], in_=ztsb[0:EMB, :])
            nc.vector.memset(ztaug[64:65, :], 1.0)
            for t in range(NT):
                pq = psS.tile([128, 4], F32, tag="psS")
                nc.tensor.matmul(out=pq[:, 0:K],
                                 lhsT=ztaug[:, t * 128:(t + 1) * 128],
                                 rhs=cent_s[0:65, :], start=True, stop=True)
                qr = spool.tile([128, K], F32, tag="qr")
                nc.vector.reciprocal(out=qr[:], in_=pq[:, 0:K])
                qs = spool.tile([128, 1], F32, tag="qs")
                nc.vector.reduce_sum(out=qs[:], in_=qr[:], axis=mybir.AxisListType.X)
                rqs = spool.tile([128, 1], F32, tag="rqs")
                nc.vector.reciprocal(out=rqs[:], in_=qs[:])
                qn = spool.tile([128, K], F32, tag="qn")
                nc.vector.tensor_scalar_mul(qn[:], qr[:], rqs[:, 0:1])
                nc.sync.dma_start(
                    out=q_out.rearrange("(t p) c -> p t c", p=128)[:, t, :], in_=qn[:])

            # adjacency: sigmoid(z_shard @ z_full^T)
            for t in range(NT):
                for r in range(NC):
                    for cc in range(NS // 512):
                        pd = psT.tile([128, 512], F32, tag="psT")
                        nc.tensor.matmul(
                            out=pd[:],
                            lhsT=ztsb[0:EMB, t * 128:(t + 1) * 128],
                            rhs=ztf[:, r * NS + cc * 512:r * NS + (cc + 1) * 512],
                            start=True, stop=True)
                        sg = epool.tile([128, 512], F32, tag="sg")
                        nc.scalar.activation(out=sg[:], in_=pd[:],
                                             func=mybir.ActivationFunctionType.Sigmoid)
                        nc.sync.dma_start(
                            out=adj_out.rearrange("(t p) c -> p t c", p=128)[
                                :, t, r * NS + cc * 512:r * NS + (cc + 1) * 512],
                            in_=sg[:])
    return nc


# ---------------- entry point ----------------

_CACHE = {}


def _get_program(ET, G):
    key = (ET, G)
    if key not in _CACHE:
        nc = bacc.Bacc("TRN2", num_devices=NC)
        _build(nc, ET, G)
        nc.compile()
        _CACHE[key] = nc
    return _CACHE[key]


def _prep_inputs(x, edge_index, W1, as1, ad1, b1, W2, as2, ad2, b2, W3, as3, ad3, b3, centers):
    x = np.asarray(x, np.float32)
    ET, G, src_pad, dst_pad, dl_pad = _prep_graph(np.asarray(edge_index, np.int64))

    import ml_dtypes
    ml_bf16 = ml_dtypes.bfloat16
    W1a = _aug_w(np.asarray(W1, np.float64), np.asarray(as1, np.float64), np.asarray(ad1, np.float64))
    W2a = _aug_w(np.asarray(W2, np.float64), np.asarray(as2, np.float64), np.asarray(ad2, np.float64))
    W3a = _aug_w(np.asarray(W3, np.float64), np.asarray(as3, np.float64), np.asarray(ad3, np.float64))
    W1b = W1a.astype(np.float32).astype(ml_bf16)
    W2b = W2a.reshape(8, 128, 1032).transpose(1, 0, 2).astype(np.float32).astype(ml_bf16)
    W3b = W3a.reshape(8, 128, 18).transpose(1, 0, 2).astype(np.float32).astype(ml_bf16)
    b1c = np.asarray(b1, np.float32).reshape(8, 128).T.copy()
    b2c = np.asarray(b2, np.float32).reshape(8, 128).T.copy()
    assert np.abs(np.asarray(b3)).max() == 0.0, "kernel assumes b3 == 0"

    cents = np.asarray(centers, np.float64)
    cent_q = np.zeros((128, K), np.float32)
    cent_q[0:EMB, :] = 1.0                       # multiplies z^2 rows
    cent_q[32:32 + EMB, :] = -2.0 * cents.T
    cent_q[64, :] = (cents ** 2).sum(axis=1) + 1.0   # |c|^2 + 1 (folds the +1 of 1/(1+d2))

    in_maps = []
    for c in range(NC):
        sh = slice(c * NS, (c + 1) * NS)
        in_maps.append({
            "xT": x[sh].T.astype(ml_bf16),
            "W1a": W1b, "W2a": W2b, "W3a": W3b,
            "b1c": b1c, "b2c": b2c, "cent": cent_q,
            "srci": _idx16(src_pad[c], G),
            "dsti": _idx16(dst_pad[c], G),
            "dli": _dlT(dl_pad[c], G),
        })
    return ET, G, in_maps


def kernel(x, edge_index, W1, as1, ad1, b1, W2, as2, ad2, b2, W3, as3, ad3, b3, centers):
    ET, G, in_maps = _prep_inputs(x, edge_index, W1, as1, ad1, b1, W2, as2, ad2, b2,
                                  W3, as3, ad3, b3, centers)
    nc = _get_program(ET, G)
    res = run_bass_kernel_spmd(nc, in_maps, core_ids=list(range(NC)))
    z = np.concatenate([res.results[c]["z_out"] for c in range(NC)], axis=0)
    adj = np.concatenate([res.results[c]["adj_out"] for c in range(NC)], axis=0)
    q = np.concatenate([res.results[c]["q_out"] for c in range(NC)], axis=0)
    return z, adj, q
